# revision 7
# baseline (speedup 1.0000x reference)
"""Multi-head causal self-attention (S=4096, D=2048, H=16) on 8 trn2 NeuronCores.

v6: fused projection+attention schedule, softmax denominator off the PE.

Sharding: tensor-parallel over heads - 2 heads per core; host sums the 8
partial out-projections (bias bo and the bv rank-1 term added on host).

Per-core structure (all matmuls bf16, PSUM f32):
  - One slot per 256-row chunk j: emit projection matmuls for chunk j, then
    attention for chunk j-1, with out-projection pieces of chunk j-2
    interleaved between attention pairs. ScalarE's exp stream and VectorE's
    denominator accumulation hide under the PE-bound projection work.
  - Denominator: exp tiles are accumulated elementwise on VectorE into two
    alternating bf16 lanes (breaking the RAW chain), folded, then a single
    ones-stationary matmul per (chunk, head) broadcasts the cross-partition
    sum into PSUM. This removes the per-pair ones-matmul (~55us of PE).
  - PSUM: one shared 3-buffer pool of [128,2,256] banks serves the three
    projection accumulators AND the score tiles (their lifetimes alternate);
    dedicated pools for the diagonal score tile, o/den bank, out-proj banks.
  - Diagonal pair: QK emitted first, exp+mask late, PV last (baseline trick).
  - The ones-matmul + reciprocal + normalize for (jc,h) are deferred into the
    next head's pair loop so the PE never waits on the DVE fold.
"""

import numpy as np
import ml_dtypes

S, D, H = 4096, 2048, 16
HD = D // H  # 128
N_CORES = 8
HPC = H // N_CORES  # heads per core = 2
DPC = HPC * HD  # head dims per core = 256
SCALE = 1.0 / np.sqrt(np.float32(HD))

C = 256  # chunk rows
NC = S // C  # 16 chunks
NDT = D // 128  # 16 d tiles
NKT = S // 128  # 32 sk tiles

_CACHE = {}


def _build():
    import concourse.bacc as bacc
    import concourse.mybir as mybir
    import concourse.tile as tile

    f32 = mybir.dt.float32
    bf = mybir.dt.bfloat16
    Exp = mybir.ActivationFunctionType.Exp
    Copy = mybir.ActivationFunctionType.Copy
    Identity = mybir.ActivationFunctionType.Identity

    nc = bacc.Bacc("TRN2", target_bir_lowering=False)

    xT = nc.dram_tensor("xT", [D, S], bf, kind="ExternalInput")
    wq = nc.dram_tensor("wq", [D, DPC], bf, kind="ExternalInput")
    wk = nc.dram_tensor("wk", [D, DPC], bf, kind="ExternalInput")
    wv = nc.dram_tensor("wv", [D, DPC], bf, kind="ExternalInput")
    wo = nc.dram_tensor("wo", [DPC, D], bf, kind="ExternalInput")
    bqk = nc.dram_tensor("bqk", [2, DPC], f32, kind="ExternalInput")
    masks = nc.dram_tensor("masks", [128, 2 * C], bf, kind="ExternalInput")
    out = nc.dram_tensor("out", [S, D], bf, kind="ExternalOutput")

    xT3 = xT.rearrange("(dt p) s -> p dt s", p=128)
    out3 = out.rearrange("(st p) d -> p st d", p=128)

    with tile.TileContext(nc) as tc:
        with (
            tc.tile_pool(name="persist", bufs=1) as persist,
            tc.tile_pool(name="xin", bufs=3) as xin,
            tc.tile_pool(name="expp", bufs=8) as expp,
            tc.tile_pool(name="lanep", bufs=2) as lanep,
            tc.tile_pool(name="accfp", bufs=2) as accfp,
            tc.tile_pool(name="otp", bufs=4) as otp,
            tc.tile_pool(name="obp", bufs=3) as obp,
            tc.tile_pool(name="rdp", bufs=2) as rdp,
            tc.tile_pool(name="pA", bufs=3, space="PSUM") as pA,
            tc.tile_pool(name="pOD", bufs=2, space="PSUM") as pOD,
            tc.tile_pool(name="pOP", bufs=2, space="PSUM") as pOP,
            tc.tile_pool(name="pDG", bufs=1, space="PSUM") as pDG,
        ):
            qT = persist.tile([128, HPC, S], bf, tag="qT")
            kT = persist.tile([128, HPC, S], bf, tag="kT")
            vn = persist.tile([128, NKT, DPC], bf, tag="vn")
            wq_sb = persist.tile([128, NDT, DPC], bf, tag="wq")
            wk_sb = persist.tile([128, NDT, DPC], bf, tag="wk")
            wv_sb = persist.tile([128, NDT, DPC], bf, tag="wv")
            wo_sb = persist.tile([128, HPC, D], bf, tag="wo")
            mask_sb = persist.tile([128, 2, C], bf, tag="mask")
            bias_sb = persist.tile([128, 2, HPC], f32, tag="bias")
            ones_bf = persist.tile([128, 128], bf, tag="ones")

            nc.vector.memset(ones_bf[:], 1.0)

            # ---- initial DMAs: minimal set for the first matmuls first ----
            xts = {}

            def dma_x(j, lo=0, hi=NDT):
                if j not in xts:
                    xts[j] = xin.tile([128, NDT, C], bf, tag="xt", name=f"xt{j}")
                nc.sync.dma_start(
                    out=xts[j][:, lo:hi, :], in_=xT3[:, lo:hi, j * C : (j + 1) * C]
                )

            def dma_w(w_sb, w_dram, lo, hi):
                nc.sync.dma_start(
                    out=w_sb[:, lo:hi, :],
                    in_=w_dram.rearrange("(dt p) m -> p dt m", p=128)[:, lo:hi, :],
                )

            dma_x(0, 0, 2)
            dma_w(wq_sb, wq, 0, 2)
            dma_x(0, 2, 4)
            dma_w(wq_sb, wq, 2, 4)
            dma_w(wk_sb, wk, 0, 4)
            dma_w(wv_sb, wv, 0, 4)
            dma_x(0, 4, NDT)
            dma_w(wq_sb, wq, 4, NDT)
            nc.sync.dma_start(
                out=bias_sb[:], in_=bqk.rearrange("b (h p) -> p b h", p=128)
            )
            dma_w(wk_sb, wk, 4, NDT)
            dma_w(wv_sb, wv, 4, NDT)
            dma_x(1)
            nc.sync.dma_start(
                out=mask_sb[:], in_=masks.rearrange("p (m n) -> p m n", m=2)
            )
            nc.sync.dma_start(
                out=wo_sb[:], in_=wo.rearrange("(h p) d -> p h d", p=128)
            )

            # ---- deferred work queues ----
            pieces = []  # out-projection piece closures
            pend_den = []  # deferred (ones-matmul + recip + normalize)

            def drain_piece():
                if pieces:
                    pieces.pop(0)()

            def drain_den():
                while pend_den:
                    pend_den.pop(0)()

            def proj(j):
                if j + 2 < NC:
                    dma_x(j + 2)
                xt = xts.pop(j)
                sq = slice(j * C, (j + 1) * C)
                # q block first, then k, then v: each accumulator's drain is
                # emitted right after its stop so it completes while the PE
                # streams the remaining blocks -> the attention score tiles
                # that reuse these PSUM banks never wait on a drain.
                ps_q = pA.tile([128, HPC, C], f32, tag="b1", name=f"psq{j}")
                for dt in range(NDT):
                    for h in range(HPC):
                        nc.tensor.matmul(
                            ps_q[:, h, :],
                            wq_sb[:, dt, h * 128 : h * 128 + 128],
                            xt[:, dt, :],
                            start=(dt == 0 and h == 0),
                            stop=(dt == NDT - 1),
                            skip_group_check=True,
                        )
                nc.scalar.activation(qT[:, 0, sq], ps_q[:, 0, :], Identity,
                                     bias=bias_sb[:, 0, 0:1])
                nc.vector.tensor_scalar_add(qT[:, 1, sq], ps_q[:, 1, :],
                                            bias_sb[:, 0, 1:2])
                ps_k = pA.tile([128, HPC, C], f32, tag="b1", name=f"psk{j}")
                for dt in range(NDT):
                    for h in range(HPC):
                        nc.tensor.matmul(
                            ps_k[:, h, :],
                            wk_sb[:, dt, h * 128 : h * 128 + 128],
                            xt[:, dt, :],
                            start=(dt == 0 and h == 0),
                            stop=(dt == NDT - 1),
                            skip_group_check=True,
                        )
                nc.scalar.activation(kT[:, 1, sq], ps_k[:, 1, :], Identity,
                                     bias=bias_sb[:, 1, 1:2])
                nc.vector.tensor_scalar_add(kT[:, 0, sq], ps_k[:, 0, :],
                                            bias_sb[:, 1, 0:1])
                ps_v = pA.tile([128, HPC, C], f32, tag="b1", name=f"psv{j}")
                for dt in range(NDT):
                    for i in range(HPC):
                        nc.tensor.matmul(
                            ps_v[:, i, :],
                            xt[:, dt, i * 128 : i * 128 + 128],
                            wv_sb[:, dt, :],
                            start=(dt == 0 and i == 0),
                            stop=(dt == NDT - 1),
                            skip_group_check=True,
                        )
                nc.vector.tensor_copy(vn[:, 2 * j : 2 * j + 2, :], ps_v[:])

            def outproj(jc, oT):
                # 8 pieces; each drains one [128,512] PSUM bank; DMA per n.
                obs = {}
                for n in range(4):
                    for si in range(2):
                        def piece(jc=jc, oT=oT, n=n, si=si):
                            if si == 0:
                                obs[n] = obp.tile([128, 2, 512], bf, tag="ob",
                                                  name=f"ob{jc}_{n}")
                            ps_p = pOP.tile([128, 512], f32, tag="op")
                            for h in range(HPC):
                                nc.tensor.matmul(
                                    ps_p[:],
                                    oT[:, h, si * 128 : si * 128 + 128],
                                    wo_sb[:, h, n * 512 : n * 512 + 512],
                                    start=(h == 0),
                                    stop=(h == HPC - 1),
                                )
                            ob = obs[n]
                            if (n + si) % 2 == 0:
                                nc.scalar.activation(ob[:, si, :], ps_p[:], Copy)
                            else:
                                nc.vector.tensor_copy(ob[:, si, :], ps_p[:])
                            if si == 1:
                                nc.sync.dma_start(
                                    out=out3[:, 2 * jc : 2 * jc + 2,
                                             n * 512 : n * 512 + 512],
                                    in_=ob[:],
                                )
                        pieces.append(piece)

            def attn(jc):
                npairs = jc + 1
                diag = npairs - 1
                others = list(range(npairs - 1))
                sq = slice(jc * C, (jc + 1) * C)
                oT = otp.tile([128, HPC, C], bf, tag="oT")
                for h in range(HPC):
                    od = pOD.tile([128, 2, C], f32, tag="od")
                    sdg = pDG.tile([128, 2, C], f32, tag="sdg")

                    def qk(pi, pool_tile):
                        for u in range(2):
                            nc.tensor.matmul(
                                pool_tile[:, u, :],
                                kT[:, h, (2 * pi + u) * 128 : (2 * pi + u) * 128 + 128],
                                qT[:, h, sq],
                                start=(u == 0),
                                stop=(u == 1),
                                skip_group_check=True,
                            )

                    def qk_exp(pi):
                        s = pA.tile([128, 2, C], f32, tag="b1", name=f"s{jc}_{h}_{pi}")
                        qk(pi, s)
                        e = expp.tile([128, 2, C], bf, tag="ex")
                        nc.scalar.activation(e[:], s[:], Exp, scale=float(SCALE))
                        return e

                    def pv(pi, e, first, last):
                        for u in range(2):
                            nc.tensor.matmul(
                                od[:, 0, :],
                                vn[:, 2 * pi + u, h * 128 : h * 128 + 128],
                                e[:, u, :],
                                start=(first and u == 0),
                                stop=(last and u == 1),
                                skip_group_check=True,
                            )

                    # --- denominator lanes (VectorE) ---
                    lanes = [None, None]
                    unpaired = [None]
                    nadd = [0]

                    def den_add(e):
                        if unpaired[0] is None and None in lanes:
                            unpaired[0] = e
                            return
                        if unpaired[0] is not None:
                            li = lanes.index(None)
                            lanes[li] = lanep.tile([128, 2, C], bf,
                                                   tag=f"lane{li}",
                                                   name=f"lane{li}_{jc}_{h}")
                            nc.vector.tensor_add(lanes[li][:], unpaired[0][:], e[:])
                            unpaired[0] = None
                        else:
                            li = nadd[0] % 2 if lanes[1] is not None else 0
                            nadd[0] += 1
                            nc.vector.tensor_add(lanes[li][:], lanes[li][:], e[:])

                    # diagonal scores first; its exp+mask late; PV last
                    qk(diag, sdg)
                    exd = None

                    def exp_diag():
                        ed = expp.tile([128, 2, C], bf, tag="ex")
                        nc.scalar.activation(ed[:], sdg[:], Exp, scale=float(SCALE))
                        edm = expp.tile([128, 2, C], bf, tag="ex")
                        nc.vector.tensor_mul(edm[:], ed[:], mask_sb[:])
                        return edm

                    if npairs == 1:
                        exd = exp_diag()

                    exq = {}
                    for pi in others[:2]:
                        exq[pi] = qk_exp(pi)

                    proc = others + [diag]
                    pend_pos = max(0, npairs - 3)
                    for i, pi in enumerate(proc):
                        if i + 2 < len(others):
                            exq[others[i + 2]] = qk_exp(others[i + 2])
                        e = exd if pi == diag else exq.pop(pi)
                        pv(pi, e, first=(i == 0), last=(i == npairs - 1))
                        den_add(e)
                        if exd is None and i == pend_pos:
                            exd = exp_diag()
                        if i == 0:
                            drain_den()
                        if i >= 1:
                            drain_piece()
                            if len(pieces) > 16:
                                drain_piece()

                    # fold lanes -> accf [128, C] bf16
                    accf = accfp.tile([128, C], bf, tag="accf")
                    if lanes[0] is None:  # npairs == 1
                        nc.vector.tensor_add(accf[:], unpaired[0][:, 0, :],
                                             unpaired[0][:, 1, :])
                    else:
                        fold_src = lanes[0]
                        if unpaired[0] is not None:
                            nc.vector.tensor_add(fold_src[:], fold_src[:],
                                                 unpaired[0][:])
                        if lanes[1] is not None:
                            nc.vector.tensor_add(fold_src[:], fold_src[:],
                                                 lanes[1][:])
                        nc.vector.tensor_add(accf[:], fold_src[:, 0, :],
                                             fold_src[:, 1, :])

                    def finish(od=od, accf=accf, h=h, oT=oT):
                        nc.tensor.matmul(od[:, 1, :], ones_bf[:], accf[:],
                                         start=False, stop=True,
                                         skip_group_check=True)
                        rd = rdp.tile([128, C], f32, tag="rd")
                        nc.vector.reciprocal_approx_fast(rd[:], od[:, 1, :])
                        nc.vector.tensor_mul(oT[:, h, :], od[:, 0, :], rd[:])

                    pend_den.append(finish)
                outproj(jc, oT)

            # ---- slot loop ----
            for j in range(NC + 1):
                if j < NC:
                    proj(j)
                if j >= 1:
                    attn(j - 1)
            drain_den()
            while pieces:
                drain_piece()
    nc.finalize()
    return nc


def _get_nc():
    if "nc" not in _CACHE:
        _CACHE["nc"] = _build()
    return _CACHE["nc"]


def _host_masks() -> np.ndarray:
    # diagonal 256x256 block: keep iff sq_col >= sk_row (+128 for upper tile)
    p = np.arange(128)[:, None]
    c = np.arange(C)[None, :]
    blocks = [(c >= p + delta).astype(np.float32) for delta in (0, 128)]
    return np.ascontiguousarray(np.concatenate(blocks, axis=1))  # [128, 512]


def make_in_maps(inputs: dict) -> list:
    bf = ml_dtypes.bfloat16
    Wq, bq = np.asarray(inputs["Wq"], np.float32), np.asarray(inputs["bq"], np.float32)
    Wk, bk = np.asarray(inputs["Wk"], np.float32), np.asarray(inputs["bk"], np.float32)
    Wv = np.asarray(inputs["Wv"], np.float32)
    Wo = np.asarray(inputs["Wo"], np.float32)
    xT = np.ascontiguousarray(
        np.asarray(inputs["hidden_states"], np.float32).T.astype(bf)
    )
    masks = _host_masks().astype(bf)
    in_maps = []
    for c in range(N_CORES):
        r = slice(c * DPC, (c + 1) * DPC)
        in_maps.append(
            {
                "xT": xT,
                "wq": np.ascontiguousarray(Wq[r, :].T.astype(bf)),
                "wk": np.ascontiguousarray(Wk[r, :].T.astype(bf)),
                "wv": np.ascontiguousarray(Wv[r, :].T.astype(bf)),
                "wo": np.ascontiguousarray(Wo[:, r].T.astype(bf)),
                "bqk": np.stack([bq[r], bk[r]]),
                "masks": masks,
            }
        )
    return in_maps


def kernel(hidden_states, Wq, bq, Wk, bk, Wv, bv, Wo, bo):
    from concourse.bass_utils import run_bass_kernel_spmd

    Wv, bv = np.asarray(Wv, np.float32), np.asarray(bv, np.float32)
    Wo, bo = np.asarray(Wo, np.float32), np.asarray(bo, np.float32)
    in_maps = make_in_maps(
        dict(hidden_states=hidden_states, Wq=Wq, bq=bq, Wk=Wk, bk=bk, Wv=Wv, Wo=Wo)
    )

    nc = _get_nc()
    results = run_bass_kernel_spmd(nc, in_maps, core_ids=list(range(N_CORES))).results

    acc = results[0]["out"].astype(np.float32)
    for c in range(1, N_CORES):
        acc += results[c]["out"].astype(np.float32)
    acc += (bo + bv @ Wo.T)[None, :]
    return acc


# revision 10
# speedup vs baseline: 1.0245x; 1.0245x over previous
"""Multi-head causal self-attention (S=4096, D=2048, H=16) on 8 trn2 NeuronCores.

v6: fused projection+attention schedule, softmax denominator off the PE.

Sharding: tensor-parallel over heads - 2 heads per core; host sums the 8
partial out-projections (bias bo and the bv rank-1 term added on host).

Per-core structure (all matmuls bf16, PSUM f32):
  - One slot per 256-row chunk j: emit projection matmuls for chunk j, then
    attention for chunk j-1, with out-projection pieces of chunk j-2
    interleaved between attention pairs. ScalarE's exp stream and VectorE's
    denominator accumulation hide under the PE-bound projection work.
  - Denominator: exp tiles are accumulated elementwise on VectorE into two
    alternating bf16 lanes (breaking the RAW chain), folded, then a single
    ones-stationary matmul per (chunk, head) broadcasts the cross-partition
    sum into PSUM. This removes the per-pair ones-matmul (~55us of PE).
  - PSUM: one shared 3-buffer pool of [128,2,256] banks serves the three
    projection accumulators AND the score tiles (their lifetimes alternate);
    dedicated pools for the diagonal score tile, o/den bank, out-proj banks.
  - Diagonal pair: QK emitted first, exp+mask late, PV last (baseline trick).
  - The ones-matmul + reciprocal + normalize for (jc,h) are deferred into the
    next head's pair loop so the PE never waits on the DVE fold.
"""

import numpy as np
import ml_dtypes

S, D, H = 4096, 2048, 16
HD = D // H  # 128
N_CORES = 8
HPC = H // N_CORES  # heads per core = 2
DPC = HPC * HD  # head dims per core = 256
SCALE = 1.0 / np.sqrt(np.float32(HD))

C = 256  # chunk rows
NC = S // C  # 16 chunks
NDT = D // 128  # 16 d tiles
NKT = S // 128  # 32 sk tiles

_CACHE = {}


def _build():
    import concourse.bacc as bacc
    import concourse.mybir as mybir
    import concourse.tile as tile

    f32 = mybir.dt.float32
    bf = mybir.dt.bfloat16
    Exp = mybir.ActivationFunctionType.Exp
    Copy = mybir.ActivationFunctionType.Copy
    Identity = mybir.ActivationFunctionType.Identity

    nc = bacc.Bacc("TRN2", target_bir_lowering=False)

    xT = nc.dram_tensor("xT", [D, S], bf, kind="ExternalInput")
    wq = nc.dram_tensor("wq", [D, DPC], bf, kind="ExternalInput")
    wk = nc.dram_tensor("wk", [D, DPC], bf, kind="ExternalInput")
    wv = nc.dram_tensor("wv", [D, DPC], bf, kind="ExternalInput")
    wo = nc.dram_tensor("wo", [DPC, D], bf, kind="ExternalInput")
    bqk = nc.dram_tensor("bqk", [2, DPC], f32, kind="ExternalInput")
    masks = nc.dram_tensor("masks", [128, 2 * C], bf, kind="ExternalInput")
    out = nc.dram_tensor("out", [S, D], bf, kind="ExternalOutput")

    xT3 = xT.rearrange("(dt p) s -> p dt s", p=128)
    out3 = out.rearrange("(st p) d -> p st d", p=128)

    with tile.TileContext(nc) as tc:
        with (
            tc.tile_pool(name="persist", bufs=1) as persist,
            tc.tile_pool(name="xin", bufs=3) as xin,
            tc.tile_pool(name="expp", bufs=8) as expp,
            tc.tile_pool(name="lanep", bufs=2) as lanep,
            tc.tile_pool(name="accfp", bufs=2) as accfp,
            tc.tile_pool(name="otp", bufs=4) as otp,
            tc.tile_pool(name="obp", bufs=3) as obp,
            tc.tile_pool(name="rdp", bufs=2) as rdp,
            tc.tile_pool(name="pA", bufs=3, space="PSUM") as pA,
            tc.tile_pool(name="pOD", bufs=2, space="PSUM") as pOD,
            tc.tile_pool(name="pOP", bufs=2, space="PSUM") as pOP,
            tc.tile_pool(name="pDG", bufs=1, space="PSUM") as pDG,
        ):
            qT = persist.tile([128, HPC, S], bf, tag="qT")
            kT = persist.tile([128, HPC, S], bf, tag="kT")
            vn = persist.tile([128, NKT, DPC], bf, tag="vn")
            wq_sb = persist.tile([128, NDT, DPC], bf, tag="wq")
            wk_sb = persist.tile([128, NDT, DPC], bf, tag="wk")
            wv_sb = persist.tile([128, NDT, DPC], bf, tag="wv")
            wo_sb = persist.tile([128, HPC, D], bf, tag="wo")
            mask_sb = persist.tile([128, 2, C], bf, tag="mask")
            bias_sb = persist.tile([128, 2, HPC], f32, tag="bias")
            ones_bf = persist.tile([128, 128], bf, tag="ones")

            nc.vector.memset(ones_bf[:], 1.0)

            # ---- initial DMAs: minimal set for the first matmuls first ----
            CP = 2 * C  # 512-wide projection chunks (8 of them)
            NP = S // CP
            xts = {}

            def dma_x(p, lo=0, hi=NDT):
                if p not in xts:
                    xts[p] = xin.tile([128, NDT, CP], bf, tag="xt", name=f"xt{p}")
                nc.sync.dma_start(
                    out=xts[p][:, lo:hi, :], in_=xT3[:, lo:hi, p * CP : (p + 1) * CP]
                )

            def dma_w(w_sb, w_dram, lo, hi):
                nc.sync.dma_start(
                    out=w_sb[:, lo:hi, :],
                    in_=w_dram.rearrange("(dt p) m -> p dt m", p=128)[:, lo:hi, :],
                )

            dma_x(0, 0, 2)
            dma_w(wq_sb, wq, 0, 2)
            dma_x(0, 2, 4)
            dma_w(wq_sb, wq, 2, 4)
            dma_w(wk_sb, wk, 0, 4)
            dma_w(wv_sb, wv, 0, 4)
            dma_x(0, 4, NDT)
            dma_w(wq_sb, wq, 4, NDT)
            nc.sync.dma_start(
                out=bias_sb[:], in_=bqk.rearrange("b (h p) -> p b h", p=128)
            )
            dma_w(wk_sb, wk, 4, NDT)
            dma_w(wv_sb, wv, 4, NDT)
            dma_x(1)
            nc.sync.dma_start(
                out=mask_sb[:], in_=masks.rearrange("p (m n) -> p m n", m=2)
            )
            nc.sync.dma_start(
                out=wo_sb[:], in_=wo.rearrange("(h p) d -> p h d", p=128)
            )

            # ---- deferred work queues ----
            pieces = []  # out-projection piece closures
            pend_den = []  # deferred (ones-matmul + recip + normalize)

            def drain_piece():
                if pieces:
                    pieces.pop(0)()

            def drain_den():
                while pend_den:
                    pend_den.pop(0)()

            def proj(p):
                # 512-wide projection chunk p = attention chunks 2p, 2p+1.
                # Six sequential 1-bank sub-blocks (q_h0,q_h1,k_h0,k_h1,v_a,
                # v_b); each drain is emitted right after its stop so it
                # completes while the PE streams the next sub-block.
                if p + 2 < NP:
                    dma_x(p + 2)
                xt = xts.pop(p)
                sq = slice(p * CP, (p + 1) * CP)
                drains = (
                    (nc.scalar.activation, 0),
                    (nc.vector.tensor_scalar_add, 1),
                )
                for b, (w_sb, dst) in enumerate(((wq_sb, qT), (wk_sb, kT))):
                    for h in range(HPC):
                        ps = pA.tile([128, 2, C], f32, tag="b1",
                                     name=f"ps{b}{h}{p}")
                        for dt in range(NDT):
                            nc.tensor.matmul(
                                ps[:],
                                w_sb[:, dt, h * 128 : h * 128 + 128],
                                xt[:, dt, :],
                                start=(dt == 0),
                                stop=(dt == NDT - 1),
                            )
                        if (b + h) % 2 == 0:
                            nc.scalar.activation(dst[:, h, sq], ps[:], Identity,
                                                 bias=bias_sb[:, b, h : h + 1])
                        else:
                            nc.vector.tensor_scalar_add(dst[:, h, sq], ps[:],
                                                        bias_sb[:, b, h : h + 1])
                for half in range(2):
                    ps_v = pA.tile([128, 2, C], f32, tag="b1",
                                   name=f"psv{half}{p}")
                    for dt in range(NDT):
                        for i in range(2):
                            t = 2 * half + i
                            nc.tensor.matmul(
                                ps_v[:, i, :],
                                xt[:, dt, t * 128 : t * 128 + 128],
                                wv_sb[:, dt, :],
                                start=(dt == 0 and i == 0),
                                stop=(dt == NDT - 1),
                                skip_group_check=True,
                            )
                    nc.vector.tensor_copy(
                        vn[:, 4 * p + 2 * half : 4 * p + 2 * half + 2, :], ps_v[:]
                    )

            def outproj(jc, oT):
                # 8 pieces; each drains one [128,512] PSUM bank; DMA per n.
                obs = {}
                for n in range(4):
                    for si in range(2):
                        def piece(jc=jc, oT=oT, n=n, si=si):
                            if si == 0:
                                obs[n] = obp.tile([128, 2, 512], bf, tag="ob",
                                                  name=f"ob{jc}_{n}")
                            ps_p = pOP.tile([128, 512], f32, tag="op")
                            for h in range(HPC):
                                nc.tensor.matmul(
                                    ps_p[:],
                                    oT[:, h, si * 128 : si * 128 + 128],
                                    wo_sb[:, h, n * 512 : n * 512 + 512],
                                    start=(h == 0),
                                    stop=(h == HPC - 1),
                                )
                            ob = obs[n]
                            if (n + si) % 2 == 0:
                                nc.scalar.activation(ob[:, si, :], ps_p[:], Copy)
                            else:
                                nc.vector.tensor_copy(ob[:, si, :], ps_p[:])
                            if si == 1:
                                nc.sync.dma_start(
                                    out=out3[:, 2 * jc : 2 * jc + 2,
                                             n * 512 : n * 512 + 512],
                                    in_=ob[:],
                                )
                        pieces.append(piece)

            def attn(jc):
                npairs = jc + 1
                diag = npairs - 1
                others = list(range(npairs - 1))
                sq = slice(jc * C, (jc + 1) * C)
                oT = otp.tile([128, HPC, C], bf, tag="oT")
                for h in range(HPC):
                    od = pOD.tile([128, 2, C], f32, tag="od")
                    sdg = pDG.tile([128, 2, C], f32, tag="sdg")

                    def qk(pi, pool_tile):
                        for u in range(2):
                            nc.tensor.matmul(
                                pool_tile[:, u, :],
                                kT[:, h, (2 * pi + u) * 128 : (2 * pi + u) * 128 + 128],
                                qT[:, h, sq],
                                start=(u == 0),
                                stop=(u == 1),
                                skip_group_check=True,
                            )

                    def qk_exp(pi):
                        s = pA.tile([128, 2, C], f32, tag="b1", name=f"s{jc}_{h}_{pi}")
                        qk(pi, s)
                        e = expp.tile([128, 2, C], bf, tag="ex")
                        nc.scalar.activation(e[:], s[:], Exp, scale=float(SCALE))
                        return e

                    def pv(pi, e, first, last):
                        for u in range(2):
                            nc.tensor.matmul(
                                od[:, 0, :],
                                vn[:, 2 * pi + u, h * 128 : h * 128 + 128],
                                e[:, u, :],
                                start=(first and u == 0),
                                stop=(last and u == 1),
                                skip_group_check=True,
                            )

                    # --- denominator lanes (VectorE) ---
                    lanes = [None, None]
                    unpaired = [None]
                    nadd = [0]

                    def den_add(e):
                        if unpaired[0] is None and None in lanes:
                            unpaired[0] = e
                            return
                        if unpaired[0] is not None:
                            li = lanes.index(None)
                            lanes[li] = lanep.tile([128, 2, C], bf,
                                                   tag=f"lane{li}",
                                                   name=f"lane{li}_{jc}_{h}")
                            nc.vector.tensor_add(lanes[li][:], unpaired[0][:], e[:])
                            unpaired[0] = None
                        else:
                            li = nadd[0] % 2 if lanes[1] is not None else 0
                            nadd[0] += 1
                            nc.vector.tensor_add(lanes[li][:], lanes[li][:], e[:])

                    # diagonal scores first; its exp+mask late; PV last
                    qk(diag, sdg)
                    exd = None

                    def exp_diag():
                        ed = expp.tile([128, 2, C], bf, tag="ex")
                        nc.scalar.activation(ed[:], sdg[:], Exp, scale=float(SCALE))
                        edm = expp.tile([128, 2, C], bf, tag="ex")
                        nc.vector.tensor_mul(edm[:], ed[:], mask_sb[:])
                        return edm

                    if npairs == 1:
                        exd = exp_diag()

                    exq = {}
                    for pi in others[:2]:
                        exq[pi] = qk_exp(pi)

                    proc = others + [diag]
                    pend_pos = max(0, npairs - 3)
                    for i, pi in enumerate(proc):
                        if i + 2 < len(others):
                            exq[others[i + 2]] = qk_exp(others[i + 2])
                        e = exd if pi == diag else exq.pop(pi)
                        pv(pi, e, first=(i == 0), last=(i == npairs - 1))
                        den_add(e)
                        if exd is None and i == pend_pos:
                            exd = exp_diag()
                        if i == 0:
                            drain_den()
                        if i >= 1:
                            drain_piece()
                            if len(pieces) > 16:
                                drain_piece()

                    # fold lanes -> accf [128, C] bf16
                    accf = accfp.tile([128, C], bf, tag="accf")
                    if lanes[0] is None:  # npairs == 1
                        nc.vector.tensor_add(accf[:], unpaired[0][:, 0, :],
                                             unpaired[0][:, 1, :])
                    else:
                        fold_src = lanes[0]
                        if unpaired[0] is not None:
                            nc.vector.tensor_add(fold_src[:], fold_src[:],
                                                 unpaired[0][:])
                        if lanes[1] is not None:
                            nc.vector.tensor_add(fold_src[:], fold_src[:],
                                                 lanes[1][:])
                        nc.vector.tensor_add(accf[:], fold_src[:, 0, :],
                                             fold_src[:, 1, :])

                    def finish(od=od, accf=accf, h=h, oT=oT):
                        nc.tensor.matmul(od[:, 1, :], ones_bf[:], accf[:],
                                         start=False, stop=True,
                                         skip_group_check=True)
                        rd = rdp.tile([128, C], f32, tag="rd")
                        nc.vector.reciprocal_approx_fast(rd[:], od[:, 1, :])
                        nc.vector.tensor_mul(oT[:, h, :], od[:, 0, :], rd[:])

                    pend_den.append(finish)
                outproj(jc, oT)

            # ---- slot loop: proj chunk p covers attention chunks 2p,2p+1;
            # keep a one-proj-chunk lag so attention reads drained tiles ----
            proj(0)
            proj(1)
            for p in range(2, NP):
                attn(2 * p - 4)
                attn(2 * p - 3)
                proj(p)
            for jc in range(2 * NP - 4, NC):
                attn(jc)
            drain_den()
            while pieces:
                drain_piece()
    nc.finalize()
    return nc


def _get_nc():
    if "nc" not in _CACHE:
        _CACHE["nc"] = _build()
    return _CACHE["nc"]


def _host_masks() -> np.ndarray:
    # diagonal 256x256 block: keep iff sq_col >= sk_row (+128 for upper tile)
    p = np.arange(128)[:, None]
    c = np.arange(C)[None, :]
    blocks = [(c >= p + delta).astype(np.float32) for delta in (0, 128)]
    return np.ascontiguousarray(np.concatenate(blocks, axis=1))  # [128, 512]


def make_in_maps(inputs: dict) -> list:
    bf = ml_dtypes.bfloat16
    Wq, bq = np.asarray(inputs["Wq"], np.float32), np.asarray(inputs["bq"], np.float32)
    Wk, bk = np.asarray(inputs["Wk"], np.float32), np.asarray(inputs["bk"], np.float32)
    Wv = np.asarray(inputs["Wv"], np.float32)
    Wo = np.asarray(inputs["Wo"], np.float32)
    xT = np.ascontiguousarray(
        np.asarray(inputs["hidden_states"], np.float32).T.astype(bf)
    )
    masks = _host_masks().astype(bf)
    in_maps = []
    for c in range(N_CORES):
        r = slice(c * DPC, (c + 1) * DPC)
        in_maps.append(
            {
                "xT": xT,
                "wq": np.ascontiguousarray(Wq[r, :].T.astype(bf)),
                "wk": np.ascontiguousarray(Wk[r, :].T.astype(bf)),
                "wv": np.ascontiguousarray(Wv[r, :].T.astype(bf)),
                "wo": np.ascontiguousarray(Wo[:, r].T.astype(bf)),
                "bqk": np.stack([bq[r], bk[r]]),
                "masks": masks,
            }
        )
    return in_maps


def kernel(hidden_states, Wq, bq, Wk, bk, Wv, bv, Wo, bo):
    from concourse.bass_utils import run_bass_kernel_spmd

    Wv, bv = np.asarray(Wv, np.float32), np.asarray(bv, np.float32)
    Wo, bo = np.asarray(Wo, np.float32), np.asarray(bo, np.float32)
    in_maps = make_in_maps(
        dict(hidden_states=hidden_states, Wq=Wq, bq=bq, Wk=Wk, bk=bk, Wv=Wv, Wo=Wo)
    )

    nc = _get_nc()
    results = run_bass_kernel_spmd(nc, in_maps, core_ids=list(range(N_CORES))).results

    acc = results[0]["out"].astype(np.float32)
    for c in range(1, N_CORES):
        acc += results[c]["out"].astype(np.float32)
    acc += (bo + bv @ Wo.T)[None, :]
    return acc
